# revision 30
# baseline (speedup 1.0000x reference)
"""Trainium2 Bass kernel for nn_BlockConv_10514079941182.

3x3 SAME conv: x[32,128,128,128] (NCHW) * kernel[128,128,3,3] (OIHW)
-> out[32,128,128,128], fp32.

Strategy: data-parallel over batch across 8 NeuronCores (4 images/core),
no collectives. Per image, x is host-padded to [C=128, 130, 130] and
held in SBUF with C_in as the partition dim. The conv is 9 accumulating
PE matmuls per 4-row output block: contraction over C_in (partition
dim), weights [C_in, C_out] stationary, shifted windows of the padded
image as the moving operand (free size 4*128=512 = one full PSUM bank).

dtype options (matmul moving/stationary; PSUM accumulates fp32 always):
- f16 (default): full PE rate AND the 2-byte FWL-eligible weight load
  hides under each matmul -> ~218 ns/matmul cadence, ~2.8e-4 rel err.
- f32r: reduced-precision fp32 (TF32-like), full PE rate at free>=256,
  ~1.4e-4 rel err, but the 4-byte per-matmul weight reload is partially
  exposed -> ~237 ns/matmul (~8% slower overall).
- f32: true fp32, 4 cycles/row (~3.6x slower). Unused.

Measured (NTFF profile, core 0): ~272-273 us HW exec for the full
per-core workload (1152 matmuls of [128x128]@[128x512]), ~91% of the
PE streaming roofline incl. fixed ~7.5us preamble + ~10.5us drain tail.
"""

import sys

for _p in ("/opt/trn_rl_repo", "/root/.axon_site/_ro/trn_rl_repo"):
    if _p not in sys.path:
        sys.path.append(_p)

import numpy as np

import concourse.bacc as bacc
import concourse.bass as bass
import concourse.mybir as mybir
import concourse.tile as tile
import concourse.bass_utils as _bu

def _dedupe_ldweights(nc):
    """With taps-outer loop order, 8 consecutive matmuls share one stationary
    tensor; delete the repeated standalone InstLdweights that tile
    legalization emits per matmul (weights persist in the PE array)."""
    n = 0
    for blk in nc.main_func.blocks:
        prev_key = None
        drop = []
        for inst in blk.instructions:
            if isinstance(inst, mybir.InstLdweights):
                si = inst.sync_info
                has_sem = si is not None and (len(si.on_wait) or len(si.on_update))
                key = (str(inst.ins[0]), str(inst.perf_mode), str(inst.tile_position))
                if key == prev_key and not has_sem:
                    drop.append(inst)
                    n += 1
                else:
                    prev_key = key
            elif isinstance(inst, mybir.InstMatmult):
                pass  # matmuls don't invalidate the loaded weights
        for inst in drop:
            blk.instructions.remove(inst)
    return n

B, C, N, K = 32, 128, 128, 3
NCORES = 8
BPC = B // NCORES  # images per core
NP = N + 2  # padded spatial size
ROWS = 4  # output rows per matmul block (4*128 = 512 free = 1 PSUM bank)
NBLK = N // ROWS

F32 = mybir.dt.float32
F32R = mybir.dt.float32r
F16 = mybir.dt.float16

_DT = {"f32r": F32R, "f32": F32, "f16": F16}
_NPDT = {"f32r": np.float32, "f32": np.float32, "f16": np.float16}


def build_nc(dtype: str = "f32r", variant: str = "v2") -> bass.Bass:
    """Build the SPMD per-core program (same on all 8 cores)."""
    nc = bacc.Bacc("TRN2", target_bir_lowering=False, debug=False)

    # float32r = reduced-precision fp32 matmul dtype: full PE rate at
    # free-dim >= 256 (vs 4x slower for true fp32), ~1.5e-4 rel err.
    # The BIR verifier requires the whole producer chain to be f32r.
    # float16: same PE rate, ~2.8e-4 rel err, but the 2-byte weight load
    # (FWL-eligible) hides under the matmul, unlike the fp32 one.
    if variant == "v9":
        xp = nc.dram_tensor("xp", [BPC, C, NP, NP], F8, kind="ExternalInput")
        wt = nc.dram_tensor("wt", [C, K * K, C], F8, kind="ExternalInput")
        out = nc.dram_tensor("out", [BPC, C, N, N], F16, kind="ExternalOutput")
        _build_v9(nc, xp, wt, out)
        nc.compile()
        return nc

    if variant == "v8":
        GRP = NBLK // 4  # groups per image (G=4 blocks each)
        xp = nc.dram_tensor("xp", [BPC, C, NP, NP], F8, kind="ExternalInput")
        wt = nc.dram_tensor("wt", [C, K * K, C], F8, kind="ExternalInput")
        s9 = nc.dram_tensor("s9", [BPC, GRP, C, 4 * ROWS * N], F16, kind="ExternalInput")
        out = nc.dram_tensor("out", [BPC, C, N, N], F32, kind="ExternalOutput")
        _build_v8(nc, xp, wt, out, s9)
        if _DEDUPE_LDW:
            _dedupe_ldweights(nc)
        nc.compile()
        return nc

    DT = _DT[dtype]
    xp = nc.dram_tensor("xp", [BPC, C, NP, NP], DT, kind="ExternalInput")
    wt = nc.dram_tensor("wt", [C, K * K, C], DT, kind="ExternalInput")
    out = nc.dram_tensor("out", [BPC, C, N, N], F32, kind="ExternalOutput")

    nc._taps_inner = variant == "v5"
    if variant == "v1":
        _build_v1(nc, xp, wt, out, DT)
    elif variant == "v2":
        _build_v2(nc, xp, wt, out, DT)
    elif variant == "v3":
        _build_v3(nc, xp, wt, out, DT)
    else:
        _build_v3(nc, xp, wt, out, DT, warmup=True, psum_tail_dma=True)
    nc.compile()
    return nc


def _build_v3(nc, xp, wt, out, DT, warmup=False, psum_tail_dma=False):
    """v2 + group-level input DMAs (4/image instead of 32) to cut the
    serial DMA-enqueue chain, a split first DMA so matmuls start after
    ~6 rows, and output DMAs alternating sync/scalar queues.
    warmup: dummy matmuls on a zeroed scratch tile during the DMA lead-in
    so the HAM clock-gate is already at 8/8 when real matmuls start.
    psum_tail_dma: DMA the final group's PSUM banks straight to DRAM,
    skipping the serial DVE copy chain in the kernel tail."""
    G = 8
    GR = G * ROWS  # 32 rows per group
    with tile.TileContext(nc) as tc:
        with (
            tc.tile_pool(name="xpool", bufs=3) as xpool,
            tc.tile_pool(name="wpool", bufs=1) as wpool,
            tc.tile_pool(name="opool", bufs=G) as opool,
            tc.tile_pool(name="pspool", bufs=G, space="PSUM") as pspool,
        ):
            wt_t = wpool.tile([C, K * K, C], DT, tag="wt")
            # weights go on sync's HWDGE queue FIRST: the gpsimd (SWDGE)
            # path measured ~4us slower start+transfer and gated the first
            # real matmul at 15us instead of ~9us
            nc.sync.dma_start(wt_t[:], wt[:])
            if warmup:
                # full-width (N=512) dummy matmuls covering the ~2us input-DMA
                # wait: they start the HAM clock-gate warm-up early without
                # delaying the first real matmul (PE runs them first in order)
                scratch = wpool.tile([C, ROWS * N], DT, tag="scratch")
                nc.gpsimd.memset(scratch[:], 0.0)
                warm_ps = pspool.tile([C, ROWS, N], F32, tag="ps", name="warm_ps")
                for _ in range(8):
                    nc.tensor.matmul(
                        warm_ps[:], scratch[:, :C], scratch[:],
                        start=True, stop=True,
                    )

            for b in range(BPC):
                for g in range(NBLK // G):
                    r0 = GR * g
                    last = b == BPC - 1 and g == NBLK // G - 1
                    xg = xpool.tile([C, GR + 2, NP], DT, tag="xg")
                    if b == 0 and g == 0:
                        # split: block 0's rows land first so the PE can start
                        nc.sync.dma_start(
                            xg[:, 0 : ROWS + 2, :], xp[0, :, 0 : ROWS + 2, :]
                        )
                        nc.sync.dma_start(
                            xg[:, ROWS + 2 :, :], xp[0, :, ROWS + 2 : GR + 2, :]
                        )
                    else:
                        nc.sync.dma_start(xg[:], xp[b, :, r0 : r0 + GR + 2, :])
                    pss = [
                        pspool.tile([C, ROWS, N], F32, tag="ps", name=f"ps{j}")
                        for j in range(G)
                    ]

                    def mm(j, t):
                        kh, kw = divmod(t, K)
                        nc.tensor.matmul(
                            pss[j][:],
                            wt_t[:, t, :],
                            xg[:, ROWS * j + kh : ROWS * j + kh + ROWS, kw : kw + N],
                            start=(t == 0),
                            stop=(t == K * K - 1),
                        )

                    def flush(j):
                        ob = opool.tile([C, ROWS, N], F32, tag="ob", name="ob")
                        nc.vector.tensor_copy(ob[:], pss[j][:])
                        # scalar dma_start = slow SWDGE (~76 GB/s): fine
                        # mid-stream where transfers hide under compute, but
                        # the final group must drain fast on sync's HWDGE or
                        # its last transfer (~3.4us) sits in the kernel tail
                        eng = nc.scalar if (j % 2 and not last) else nc.sync
                        eng.dma_start(
                            out[b, :, r0 + ROWS * j : r0 + ROWS * (j + 1), :], ob[:]
                        )

                    if (last and psum_tail_dma) or nc._taps_inner:
                        # taps-inner: each bank finishes (and flushes) early;
                        # only the final block's copy+DMA lands in the tail
                        for j in range(G):
                            for t in range(K * K):
                                mm(j, t)
                            flush(j)
                    else:
                        for t in range(K * K):
                            for j in range(G):
                                mm(j, t)
                        for j in range(G):
                            flush(j)


F8 = mybir.dt.float8e4
_DEDUPE_LDW = False
XS = 16.0  # x fp8 scale
WS = 256.0  # w' fp8 scale
DESCALE = 1.0 / (XS * WS)

# tap pairing for DoubleRow: taps row-major (kh*3+kw); pairs (0,1),(2,3),
# (4,5),(6,7), solo tap 8. delta = SBUF element offset between the pair's
# shifted windows of the padded image tile.
_PAIRS = [(0, 1), (2, 3), (4, 5), (6, 7)]
_SOLO = 8


def _pair_delta(t0, t1):
    kh0, kw0 = divmod(t0, K)
    kh1, kw1 = divmod(t1, K)
    return (kh1 - kh0) * NP + (kw1 - kw0)


def _dr_rhs(xg, j, t0, delta):
    """Moving AP [C, 2, ROWS, N]: pair axis = two shifted windows (stride
    delta) of the padded group tile xg[C, GR+2, NP]."""
    kh0, kw0 = divmod(t0, K)
    ap = xg[:, ROWS * j + kh0 : ROWS * j + kh0 + ROWS, kw0 : kw0 + N].unsqueeze(1)
    v = ap.ap
    V = type(v)
    dims = [list(d) for d in v]
    dims[1] = [delta, 2]
    ap.ap = V(dims)
    return ap


def _build_v8(nc, xp, wt, out, s9, warmup=True, G=4):
    """fp8e4 DoubleRow conv: per block 4 DR pair-matmuls + 1 solo fp8 tap,
    weights mean-subtracted on host; the 0.5*box9(channel-sum) correction
    S is host-replicated and added in the DVE flush (scalar_tensor_tensor:
    out = psum*DESCALE + S). Groups of G=4 blocks alternate between the two
    halves of the 8 PSUM banks so a group's flushes overlap the next
    group's matmuls instead of stalling its start=True taps."""
    from concourse.tile_rust import add_dep_helper

    GR = G * ROWS  # rows per group
    F16 = mybir.dt.float16
    DR = mybir.MatmulPerfMode.DoubleRow
    deltas = [_pair_delta(t0, t1) for t0, t1 in _PAIRS]
    chain = [None]

    def _chain(bi):
        # total order over all matmuls: keeps the scheduler in taps-outer
        # order so consecutive matmuls share one weight load
        if chain[0] is not None:
            add_dep_helper(bi.ins, chain[0].ins, reason="taps-outer order")
        chain[0] = bi
    with tile.TileContext(nc) as tc:
        with (
            tc.tile_pool(name="xpool", bufs=3) as xpool,
            tc.tile_pool(name="wpool", bufs=1) as wpool,
            tc.tile_pool(name="srpool", bufs=3) as srpool,
            tc.tile_pool(name="opool", bufs=8) as opool,
            tc.tile_pool(name="pspool", bufs=8, space="PSUM") as pspool,
        ):
            wt_t = wpool.tile([C, K * K, C], F8, tag="wt")
            nc.sync.dma_start(wt_t[:], wt[:])
            if warmup:
                scratch = wpool.tile([C, ROWS * N], F8, tag="scratch")
                nc.gpsimd.memset(scratch[:], 0.0)
                warm_ps = pspool.tile([C, ROWS, N], F32, tag="ps", name="warm_ps")
                for _ in range(8):
                    nc.tensor.matmul(
                        warm_ps[:], scratch[:, :C], scratch[:],
                        start=True, stop=True,
                    )

            for b in range(BPC):
                for g in range(NBLK // G):
                    r0 = GR * g
                    xg = xpool.tile([C, GR + 2, NP], F8, tag="xg")
                    if b == 0 and g == 0:
                        nc.sync.dma_start(
                            xg[:, 0 : ROWS + 2, :], xp[0, :, 0 : ROWS + 2, :]
                        )
                        nc.sync.dma_start(
                            xg[:, ROWS + 2 :, :], xp[0, :, ROWS + 2 : GR + 2, :]
                        )
                    else:
                        nc.sync.dma_start(xg[:], xp[b, :, r0 : r0 + GR + 2, :])
                    # S correction for this group, host-replicated across
                    # partitions in DRAM: straight [128, GR*N] DMA
                    srep = srpool.tile([C, GR * N], F16, tag="srep")
                    nc.gpsimd.dma_start(srep[:], s9[b, g])

                    pss = [
                        pspool.tile([C, ROWS, N], F32, tag="ps", name=f"ps{j}")
                        for j in range(G)
                    ]

                    last = b == BPC - 1 and g == NBLK // G - 1
                    kh_s, kw_s = divmod(_SOLO, K)

                    def mm(j, p):
                        if p < len(_PAIRS):
                            t0 = _PAIRS[p][0]
                            bi = nc.tensor.matmul(
                                pss[j][:],
                                wt_t[:, t0 : t0 + 2, :],
                                _dr_rhs(xg, j, t0, deltas[p]),
                                start=(p == 0),
                                stop=False,
                                perf_mode=DR,
                            )
                        else:
                            bi = nc.tensor.matmul(
                                pss[j][:],
                                wt_t[:, _SOLO, :],
                                xg[
                                    :,
                                    ROWS * j + kh_s : ROWS * j + kh_s + ROWS,
                                    kw_s : kw_s + N,
                                ],
                                start=False,
                                stop=True,
                            )
                        _chain(bi)

                    def flush(j):
                        ob = opool.tile([C, ROWS, N], F32, tag="ob", name="ob")
                        nc.vector.scalar_tensor_tensor(
                            ob[:],
                            pss[j][:],
                            DESCALE,
                            srep[:, ROWS * N * j : ROWS * N * (j + 1)],
                            op0=mybir.AluOpType.mult,
                            op1=mybir.AluOpType.add,
                        )
                        eng = nc.scalar if (j % 2 and not last) else nc.sync
                        eng.dma_start(
                            out[b, :, r0 + ROWS * j : r0 + ROWS * (j + 1), :], ob[:]
                        )

                    if last:
                        # taps-inner: each block finishes (and flushes) early
                        # so only the final block's flush+DMA sits in the tail
                        for j in range(G):
                            for p in range(len(_PAIRS) + 1):
                                mm(j, p)
                            flush(j)
                    else:
                        for p in range(len(_PAIRS) + 1):
                            for j in range(G):
                                mm(j, p)
                        for j in range(G):
                            flush(j)


def _build_v9(nc, xp, wt, out, warmup=True, G=8):
    """fp8e4 DoubleRow conv, mean-correction moved to host post-processing.
    Per block: 4 DR pair-matmuls + 1 solo fp8 tap into one PSUM bank, then a
    single-input descale flush (out_f16 = psum * 2^-12) alternating between
    the Vector and Scalar engines so banks drain ~2x faster than the PE
    consumes them. Taps-outer order is forced so 8 consecutive matmuls share
    one DoubleRow weight load (stream rate ~222ns/mm)."""
    from concourse.tile_rust import add_dep_helper

    GR = G * ROWS
    F16 = mybir.dt.float16
    DR = mybir.MatmulPerfMode.DoubleRow
    deltas = [_pair_delta(t0, t1) for t0, t1 in _PAIRS]
    chain = [None]

    def _chain(bi):
        if chain[0] is not None:
            add_dep_helper(bi.ins, chain[0].ins, reason="taps-outer order")
        chain[0] = bi

    with tile.TileContext(nc) as tc:
        with (
            tc.tile_pool(name="xpool", bufs=3) as xpool,
            tc.tile_pool(name="wpool", bufs=1) as wpool,
            tc.tile_pool(name="opool", bufs=2 * G) as opool,
            tc.tile_pool(name="pspool", bufs=8, space="PSUM") as pspool,
        ):
            wt_t = wpool.tile([C, K * K, C], F8, tag="wt")
            nc.sync.dma_start(wt_t[:], wt[:])
            if warmup:
                scratch = wpool.tile([C, ROWS * N], F8, tag="scratch")
                nc.gpsimd.memset(scratch[:], 0.0)
                warm_ps = pspool.tile([C, ROWS, N], F32, tag="ps", name="warm_ps")
                for _ in range(8):
                    nc.tensor.matmul(
                        warm_ps[:], scratch[:, :C], scratch[:],
                        start=True, stop=True,
                    )

            for b in range(BPC):
                for g in range(NBLK // G):
                    r0 = GR * g
                    xg = xpool.tile([C, GR + 2, NP], F8, tag="xg")
                    if b == 0 and g == 0:
                        nc.sync.dma_start(
                            xg[:, 0 : ROWS + 2, :], xp[0, :, 0 : ROWS + 2, :]
                        )
                        nc.sync.dma_start(
                            xg[:, ROWS + 2 :, :], xp[0, :, ROWS + 2 : GR + 2, :]
                        )
                    else:
                        nc.sync.dma_start(xg[:], xp[b, :, r0 : r0 + GR + 2, :])

                    pss = [
                        pspool.tile([C, ROWS, N], F32, tag="ps", name=f"ps{j}")
                        for j in range(G)
                    ]
                    last = b == BPC - 1 and g == NBLK // G - 1
                    kh_s, kw_s = divmod(_SOLO, K)

                    def mm(j, p):
                        if p < len(_PAIRS):
                            t0 = _PAIRS[p][0]
                            bi = nc.tensor.matmul(
                                pss[j][:],
                                wt_t[:, t0 : t0 + 2, :],
                                _dr_rhs(xg, j, t0, deltas[p]),
                                start=(p == 0),
                                stop=False,
                                perf_mode=DR,
                            )
                        else:
                            bi = nc.tensor.matmul(
                                pss[j][:],
                                wt_t[:, _SOLO, :],
                                xg[
                                    :,
                                    ROWS * j + kh_s : ROWS * j + kh_s + ROWS,
                                    kw_s : kw_s + N,
                                ],
                                start=False,
                                stop=True,
                            )
                        _chain(bi)

                    def flush(j):
                        # split the bank drain across DVE (rows 0-1) and ACT
                        # (rows 2-3) so the bank frees in ~350ns instead of
                        # ~600ns serialized behind 3 other flushes
                        h = ROWS // 2
                        ob = opool.tile([C, ROWS, N], F16, tag="ob", name="ob")
                        nc.vector.tensor_scalar_mul(
                            ob[:, :h, :], pss[j][:, :h, :], DESCALE
                        )
                        nc.scalar.activation(
                            ob[:, h:, :],
                            pss[j][:, h:, :],
                            mybir.ActivationFunctionType.Identity,
                            scale=DESCALE,
                        )
                        eng = nc.scalar if (j % 2 and not last) else nc.sync
                        eng.dma_start(
                            out[b, :, r0 + ROWS * j : r0 + ROWS * (j + 1), :], ob[:]
                        )

                    if last:
                        for j in range(G):
                            for p in range(len(_PAIRS) + 1):
                                mm(j, p)
                            flush(j)
                    else:
                        for p in range(len(_PAIRS) + 1):
                            for j in range(G):
                                mm(j, p)
                        for j in range(G):
                            flush(j)


def _build_v1(nc, xp, wt, out, DT):
    """Whole-image input tiles; taps inner per block."""
    with tile.TileContext(nc) as tc:
        with (
            tc.tile_pool(name="xpool", bufs=2) as xpool,
            tc.tile_pool(name="wpool", bufs=1) as wpool,
            tc.tile_pool(name="opool", bufs=4) as opool,
            tc.tile_pool(name="pspool", bufs=8, space="PSUM") as pspool,
        ):
            wt_t = wpool.tile([C, K * K, C], DT, tag="wt")
            nc.sync.dma_start(wt_t[:], wt[:])

            for b in range(BPC):
                xp_t = xpool.tile([C, NP, NP], DT, tag="xp")
                nc.sync.dma_start(xp_t[:], xp[b])

                for r in range(NBLK):
                    ps = pspool.tile([C, ROWS, N], F32, tag="ps")
                    for t in range(K * K):
                        kh, kw = divmod(t, K)
                        rhs = xp_t[:, ROWS * r + kh : ROWS * r + kh + ROWS, kw : kw + N]
                        nc.tensor.matmul(
                            ps[:], wt_t[:, t, :], rhs,
                            start=(t == 0), stop=(t == K * K - 1),
                        )
                    ob = opool.tile([C, ROWS, N], F32, tag="ob")
                    nc.vector.tensor_copy(ob[:], ps[:])
                    nc.sync.dma_start(out[b, :, ROWS * r : ROWS * (r + 1), :], ob[:])


def _build_v2(nc, xp, wt, out, DT):
    """Per-block input tiles (ROWS+2 padded rows incl. halo) so compute
    starts after one small DMA; taps outer over groups of 8 blocks so 8
    consecutive matmuls share one weight load across 8 PSUM banks."""
    G = 8  # blocks per group = PSUM banks
    with tile.TileContext(nc) as tc:
        with (
            tc.tile_pool(name="xpool", bufs=2 * G) as xpool,
            tc.tile_pool(name="wpool", bufs=1) as wpool,
            tc.tile_pool(name="opool", bufs=G) as opool,
            tc.tile_pool(name="pspool", bufs=G, space="PSUM") as pspool,
        ):
            wt_t = wpool.tile([C, K * K, C], DT, tag="wt")
            nc.sync.dma_start(wt_t[:], wt[:])

            for b in range(BPC):
                for g in range(NBLK // G):
                    xb = []
                    for j in range(G):
                        r = g * G + j
                        xt = xpool.tile([C, ROWS + 2, NP], DT, tag="xb")
                        nc.sync.dma_start(
                            xt[:], xp[b, :, ROWS * r : ROWS * r + ROWS + 2, :]
                        )
                        xb.append(xt)
                    pss = [
                        pspool.tile([C, ROWS, N], F32, tag="ps", name=f"ps{j}")
                        for j in range(G)
                    ]
                    for t in range(K * K):
                        kh, kw = divmod(t, K)
                        for j in range(G):
                            nc.tensor.matmul(
                                pss[j][:],
                                wt_t[:, t, :],
                                xb[j][:, kh : kh + ROWS, kw : kw + N],
                                start=(t == 0),
                                stop=(t == K * K - 1),
                            )
                    for j in range(G):
                        r = g * G + j
                        ob = opool.tile([C, ROWS, N], F32, tag="ob")
                        nc.vector.tensor_copy(ob[:], pss[j][:])
                        nc.sync.dma_start(
                            out[b, :, ROWS * r : ROWS * (r + 1), :], ob[:]
                        )


def prep_inputs_v9(x: np.ndarray, kernel: np.ndarray):
    """fp8 prep without device-side S: returns (in_maps, s9) where s9 is the
    host-side mean-correction map 0.5*box9(channel-sum of x), [B, N, N] f32."""
    import ml_dtypes

    E4 = ml_dtypes.float8_e4m3
    x = np.asarray(x)
    kernel = np.asarray(kernel)
    xpad = np.zeros((B, C, NP, NP), E4)
    xpad[:, :, 1 : N + 1, 1 : N + 1] = np.clip(x * XS, -240, 240).astype(E4)
    wq = np.ascontiguousarray(
        ((kernel - 0.5) * WS).transpose(1, 2, 3, 0).reshape(C, K * K, C).astype(E4)
    )
    T = x.sum(axis=1, dtype=np.float32)  # [B, N, N]
    Tp = np.zeros((B, NP, NP), np.float32)
    Tp[:, 1 : N + 1, 1 : N + 1] = T
    s9 = np.zeros((B, N, N), np.float32)
    for kh in range(K):
        for kw in range(K):
            s9 += Tp[:, kh : kh + N, kw : kw + N]
    s9 *= 0.5
    in_maps = [
        {"xp": np.ascontiguousarray(xpad[i * BPC : (i + 1) * BPC]), "wt": wq}
        for i in range(NCORES)
    ]
    return in_maps, s9


def prep_inputs_v8(x: np.ndarray, kernel: np.ndarray):
    """fp8 prep: quantize x (scale 16) and mean-subtracted kernel (scale 256)
    to e4m3; precompute S = 0.5*box9(channel-sum of x) as f16."""
    import ml_dtypes

    E4 = ml_dtypes.float8_e4m3
    x = np.asarray(x)
    kernel = np.asarray(kernel)
    xpad = np.zeros((B, C, NP, NP), E4)
    xpad[:, :, 1 : N + 1, 1 : N + 1] = np.clip(x * XS, -240, 240).astype(E4)
    wq = np.ascontiguousarray(
        ((kernel - 0.5) * WS).transpose(1, 2, 3, 0).reshape(C, K * K, C).astype(E4)
    )
    T = x.sum(axis=1, dtype=np.float32)  # [B, N, N]
    Tp = np.zeros((B, NP, NP), np.float32)
    Tp[:, 1 : N + 1, 1 : N + 1] = T
    s9 = np.zeros((B, N, N), np.float32)
    for kh in range(K):
        for kw in range(K):
            s9 += Tp[:, kh : kh + N, kw : kw + N]
    GRP = N // 16  # groups of 16 rows per image
    s9 = (0.5 * s9).astype(np.float16).reshape(B, GRP, 1, 16 * N)
    s9r = np.ascontiguousarray(np.broadcast_to(s9, (B, GRP, C, 16 * N)))
    return [
        {
            "xp": np.ascontiguousarray(xpad[i * BPC : (i + 1) * BPC]),
            "wt": wq,
            "s9": s9r[i * BPC : (i + 1) * BPC],
        }
        for i in range(NCORES)
    ]


def prep_inputs(x: np.ndarray, kernel: np.ndarray, dtype: str = "f32r"):
    """Host-side prep: zero-pad x spatially, transpose kernel to [C_in, tap, C_out]."""
    npdt = _NPDT[dtype]
    x = np.asarray(x)
    kernel = np.asarray(kernel)
    xpad = np.zeros((B, C, NP, NP), dtype=npdt)
    xpad[:, :, 1 : N + 1, 1 : N + 1] = x
    # wt[c, kh*K+kw, o] = kernel[o, c, kh, kw]
    wt = np.ascontiguousarray(
        kernel.transpose(1, 2, 3, 0).reshape(C, K * K, C).astype(npdt)
    )
    in_maps = []
    for i in range(NCORES):
        in_maps.append(
            {
                "xp": np.ascontiguousarray(xpad[i * BPC : (i + 1) * BPC]),
                "wt": wt,
            }
        )
    return in_maps


def run(
    x: np.ndarray,
    kernel: np.ndarray,
    trace: bool = False,
    dtype: str = "f16",
    tmpdir: str | None = None,
    variant: str = "v4",
):
    """Build, compile, run on 8 cores; returns (out, BassKernelResults)."""
    from concourse.bass_utils import run_bass_kernel_spmd

    nc = build_nc(dtype=dtype, variant=variant)
    s9 = None
    if variant == "v9":
        in_maps, s9 = prep_inputs_v9(x, kernel)
    elif variant == "v8":
        in_maps = prep_inputs_v8(x, kernel)
    else:
        in_maps = prep_inputs(x, kernel, dtype=dtype)
    res = run_bass_kernel_spmd(
        nc, in_maps, core_ids=list(range(NCORES)), trace=trace, tmpdir=tmpdir
    )
    out = np.concatenate([res.results[i]["out"] for i in range(NCORES)], axis=0)
    if s9 is not None:
        # add back the host-side mean-correction (w = (w-0.5) + 0.5 split)
        out = out.astype(np.float32) + s9[:, None, :, :]
    return out, res


def kernel(x: np.ndarray, kernel: np.ndarray) -> np.ndarray:
    out, _ = run(x, kernel, trace=False, dtype="f16", variant="v4")
    return out



# revision 31
# speedup vs baseline: 1.1648x; 1.1648x over previous
"""Trainium2 Bass kernel for nn_BlockConv_10514079941182.

3x3 SAME conv: x[32,128,128,128] (NCHW) * kernel[128,128,3,3] (OIHW)
-> out[32,128,128,128], fp32.

Strategy: data-parallel over batch across 8 NeuronCores (4 images/core),
no collectives. Per image, x is host-padded to [C=128, 130, 130] and
held in SBUF with C_in as the partition dim. The conv is 9 accumulating
PE matmuls per 4-row output block: contraction over C_in (partition
dim), weights [C_in, C_out] stationary, shifted windows of the padded
image as the moving operand (free size 4*128=512 = one full PSUM bank).

dtype options (matmul moving/stationary; PSUM accumulates fp32 always):
- f16 (default): full PE rate AND the 2-byte FWL-eligible weight load
  hides under each matmul -> ~218 ns/matmul cadence, ~2.8e-4 rel err.
- f32r: reduced-precision fp32 (TF32-like), full PE rate at free>=256,
  ~1.4e-4 rel err, but the 4-byte per-matmul weight reload is partially
  exposed -> ~237 ns/matmul (~8% slower overall).
- f32: true fp32, 4 cycles/row (~3.6x slower). Unused.

Measured (NTFF profile, core 0): ~272-273 us HW exec for the full
per-core workload (1152 matmuls of [128x128]@[128x512]), ~91% of the
PE streaming roofline incl. fixed ~7.5us preamble + ~10.5us drain tail.
"""

import sys

for _p in ("/opt/trn_rl_repo", "/root/.axon_site/_ro/trn_rl_repo"):
    if _p not in sys.path:
        sys.path.append(_p)

import numpy as np

import concourse.bacc as bacc
import concourse.bass as bass
import concourse.mybir as mybir
import concourse.tile as tile
import concourse.bass_utils as _bu

def _dedupe_ldweights(nc):
    """With taps-outer loop order, 8 consecutive matmuls share one stationary
    tensor; delete the repeated standalone InstLdweights that tile
    legalization emits per matmul (weights persist in the PE array)."""
    n = 0
    for blk in nc.main_func.blocks:
        prev_key = None
        drop = []
        for inst in blk.instructions:
            if isinstance(inst, mybir.InstLdweights):
                si = inst.sync_info
                has_sem = si is not None and (len(si.on_wait) or len(si.on_update))
                key = (str(inst.ins[0]), str(inst.perf_mode), str(inst.tile_position))
                if key == prev_key and not has_sem:
                    drop.append(inst)
                    n += 1
                else:
                    prev_key = key
            elif isinstance(inst, mybir.InstMatmult):
                pass  # matmuls don't invalidate the loaded weights
        for inst in drop:
            blk.instructions.remove(inst)
    return n

B, C, N, K = 32, 128, 128, 3
NCORES = 8
BPC = B // NCORES  # images per core
NP = N + 2  # padded spatial size
ROWS = 4  # output rows per matmul block (4*128 = 512 free = 1 PSUM bank)
NBLK = N // ROWS

F32 = mybir.dt.float32
F32R = mybir.dt.float32r
F16 = mybir.dt.float16

_DT = {"f32r": F32R, "f32": F32, "f16": F16}
_NPDT = {"f32r": np.float32, "f32": np.float32, "f16": np.float16}


def build_nc(dtype: str = "f32r", variant: str = "v2") -> bass.Bass:
    """Build the SPMD per-core program (same on all 8 cores)."""
    nc = bacc.Bacc("TRN2", target_bir_lowering=False, debug=False)

    # float32r = reduced-precision fp32 matmul dtype: full PE rate at
    # free-dim >= 256 (vs 4x slower for true fp32), ~1.5e-4 rel err.
    # The BIR verifier requires the whole producer chain to be f32r.
    # float16: same PE rate, ~2.8e-4 rel err, but the 2-byte weight load
    # (FWL-eligible) hides under the matmul, unlike the fp32 one.
    if variant == "v9":
        xp = nc.dram_tensor("xp", [BPC, C, NP, NP], F8, kind="ExternalInput")
        wt = nc.dram_tensor("wt", [C, K * K, C], F8, kind="ExternalInput")
        out = nc.dram_tensor("out", [BPC, C, N, N], F16, kind="ExternalOutput")
        _build_v9(nc, xp, wt, out)
        nc.compile()
        return nc

    if variant == "v8":
        GRP = NBLK // 4  # groups per image (G=4 blocks each)
        xp = nc.dram_tensor("xp", [BPC, C, NP, NP], F8, kind="ExternalInput")
        wt = nc.dram_tensor("wt", [C, K * K, C], F8, kind="ExternalInput")
        s9 = nc.dram_tensor("s9", [BPC, GRP, C, 4 * ROWS * N], F16, kind="ExternalInput")
        out = nc.dram_tensor("out", [BPC, C, N, N], F32, kind="ExternalOutput")
        _build_v8(nc, xp, wt, out, s9)
        if _DEDUPE_LDW:
            _dedupe_ldweights(nc)
        nc.compile()
        return nc

    DT = _DT[dtype]
    xp = nc.dram_tensor("xp", [BPC, C, NP, NP], DT, kind="ExternalInput")
    wt = nc.dram_tensor("wt", [C, K * K, C], DT, kind="ExternalInput")
    out = nc.dram_tensor("out", [BPC, C, N, N], F32, kind="ExternalOutput")

    nc._taps_inner = variant == "v5"
    if variant == "v1":
        _build_v1(nc, xp, wt, out, DT)
    elif variant == "v2":
        _build_v2(nc, xp, wt, out, DT)
    elif variant == "v3":
        _build_v3(nc, xp, wt, out, DT)
    else:
        _build_v3(nc, xp, wt, out, DT, warmup=True, psum_tail_dma=True)
    nc.compile()
    return nc


def _build_v3(nc, xp, wt, out, DT, warmup=False, psum_tail_dma=False):
    """v2 + group-level input DMAs (4/image instead of 32) to cut the
    serial DMA-enqueue chain, a split first DMA so matmuls start after
    ~6 rows, and output DMAs alternating sync/scalar queues.
    warmup: dummy matmuls on a zeroed scratch tile during the DMA lead-in
    so the HAM clock-gate is already at 8/8 when real matmuls start.
    psum_tail_dma: DMA the final group's PSUM banks straight to DRAM,
    skipping the serial DVE copy chain in the kernel tail."""
    G = 8
    GR = G * ROWS  # 32 rows per group
    with tile.TileContext(nc) as tc:
        with (
            tc.tile_pool(name="xpool", bufs=3) as xpool,
            tc.tile_pool(name="wpool", bufs=1) as wpool,
            tc.tile_pool(name="opool", bufs=G) as opool,
            tc.tile_pool(name="pspool", bufs=G, space="PSUM") as pspool,
        ):
            wt_t = wpool.tile([C, K * K, C], DT, tag="wt")
            # weights go on sync's HWDGE queue FIRST: the gpsimd (SWDGE)
            # path measured ~4us slower start+transfer and gated the first
            # real matmul at 15us instead of ~9us
            nc.sync.dma_start(wt_t[:], wt[:])
            if warmup:
                # full-width (N=512) dummy matmuls covering the ~2us input-DMA
                # wait: they start the HAM clock-gate warm-up early without
                # delaying the first real matmul (PE runs them first in order)
                scratch = wpool.tile([C, ROWS * N], DT, tag="scratch")
                nc.gpsimd.memset(scratch[:], 0.0)
                warm_ps = pspool.tile([C, ROWS, N], F32, tag="ps", name="warm_ps")
                for _ in range(8):
                    nc.tensor.matmul(
                        warm_ps[:], scratch[:, :C], scratch[:],
                        start=True, stop=True,
                    )

            for b in range(BPC):
                for g in range(NBLK // G):
                    r0 = GR * g
                    last = b == BPC - 1 and g == NBLK // G - 1
                    xg = xpool.tile([C, GR + 2, NP], DT, tag="xg")
                    if b == 0 and g == 0:
                        # split: block 0's rows land first so the PE can start
                        nc.sync.dma_start(
                            xg[:, 0 : ROWS + 2, :], xp[0, :, 0 : ROWS + 2, :]
                        )
                        nc.sync.dma_start(
                            xg[:, ROWS + 2 :, :], xp[0, :, ROWS + 2 : GR + 2, :]
                        )
                    else:
                        nc.sync.dma_start(xg[:], xp[b, :, r0 : r0 + GR + 2, :])
                    pss = [
                        pspool.tile([C, ROWS, N], F32, tag="ps", name=f"ps{j}")
                        for j in range(G)
                    ]

                    def mm(j, t):
                        kh, kw = divmod(t, K)
                        nc.tensor.matmul(
                            pss[j][:],
                            wt_t[:, t, :],
                            xg[:, ROWS * j + kh : ROWS * j + kh + ROWS, kw : kw + N],
                            start=(t == 0),
                            stop=(t == K * K - 1),
                        )

                    def flush(j):
                        ob = opool.tile([C, ROWS, N], F32, tag="ob", name="ob")
                        nc.vector.tensor_copy(ob[:], pss[j][:])
                        # scalar dma_start = slow SWDGE (~76 GB/s): fine
                        # mid-stream where transfers hide under compute, but
                        # the final group must drain fast on sync's HWDGE or
                        # its last transfer (~3.4us) sits in the kernel tail
                        eng = nc.scalar if (j % 2 and not last) else nc.sync
                        eng.dma_start(
                            out[b, :, r0 + ROWS * j : r0 + ROWS * (j + 1), :], ob[:]
                        )

                    if (last and psum_tail_dma) or nc._taps_inner:
                        # taps-inner: each bank finishes (and flushes) early;
                        # only the final block's copy+DMA lands in the tail
                        for j in range(G):
                            for t in range(K * K):
                                mm(j, t)
                            flush(j)
                    else:
                        for t in range(K * K):
                            for j in range(G):
                                mm(j, t)
                        for j in range(G):
                            flush(j)


F8 = mybir.dt.float8e4
_DEDUPE_LDW = False
XS = 16.0  # x fp8 scale
WS = 256.0  # w' fp8 scale
DESCALE = 1.0 / (XS * WS)

# tap pairing for DoubleRow: taps row-major (kh*3+kw); pairs (0,1),(2,3),
# (4,5),(6,7), solo tap 8. delta = SBUF element offset between the pair's
# shifted windows of the padded image tile.
_PAIRS = [(0, 1), (2, 3), (4, 5), (6, 7)]
_SOLO = 8


def _pair_delta(t0, t1):
    kh0, kw0 = divmod(t0, K)
    kh1, kw1 = divmod(t1, K)
    return (kh1 - kh0) * NP + (kw1 - kw0)


def _dr_rhs(xg, j, t0, delta):
    """Moving AP [C, 2, ROWS, N]: pair axis = two shifted windows (stride
    delta) of the padded group tile xg[C, GR+2, NP]."""
    kh0, kw0 = divmod(t0, K)
    ap = xg[:, ROWS * j + kh0 : ROWS * j + kh0 + ROWS, kw0 : kw0 + N].unsqueeze(1)
    v = ap.ap
    V = type(v)
    dims = [list(d) for d in v]
    dims[1] = [delta, 2]
    ap.ap = V(dims)
    return ap


def _build_v8(nc, xp, wt, out, s9, warmup=True, G=4):
    """fp8e4 DoubleRow conv: per block 4 DR pair-matmuls + 1 solo fp8 tap,
    weights mean-subtracted on host; the 0.5*box9(channel-sum) correction
    S is host-replicated and added in the DVE flush (scalar_tensor_tensor:
    out = psum*DESCALE + S). Groups of G=4 blocks alternate between the two
    halves of the 8 PSUM banks so a group's flushes overlap the next
    group's matmuls instead of stalling its start=True taps."""
    from concourse.tile_rust import add_dep_helper

    GR = G * ROWS  # rows per group
    F16 = mybir.dt.float16
    DR = mybir.MatmulPerfMode.DoubleRow
    deltas = [_pair_delta(t0, t1) for t0, t1 in _PAIRS]
    chain = [None]

    def _chain(bi):
        # total order over all matmuls: keeps the scheduler in taps-outer
        # order so consecutive matmuls share one weight load
        if chain[0] is not None:
            add_dep_helper(bi.ins, chain[0].ins, reason="taps-outer order")
        chain[0] = bi
    with tile.TileContext(nc) as tc:
        with (
            tc.tile_pool(name="xpool", bufs=3) as xpool,
            tc.tile_pool(name="wpool", bufs=1) as wpool,
            tc.tile_pool(name="srpool", bufs=3) as srpool,
            tc.tile_pool(name="opool", bufs=8) as opool,
            tc.tile_pool(name="pspool", bufs=8, space="PSUM") as pspool,
        ):
            wt_t = wpool.tile([C, K * K, C], F8, tag="wt")
            nc.sync.dma_start(wt_t[:], wt[:])
            if warmup:
                scratch = wpool.tile([C, ROWS * N], F8, tag="scratch")
                nc.gpsimd.memset(scratch[:], 0.0)
                warm_ps = pspool.tile([C, ROWS, N], F32, tag="ps", name="warm_ps")
                for _ in range(8):
                    nc.tensor.matmul(
                        warm_ps[:], scratch[:, :C], scratch[:],
                        start=True, stop=True,
                    )

            for b in range(BPC):
                for g in range(NBLK // G):
                    r0 = GR * g
                    xg = xpool.tile([C, GR + 2, NP], F8, tag="xg")
                    if b == 0 and g == 0:
                        nc.sync.dma_start(
                            xg[:, 0 : ROWS + 2, :], xp[0, :, 0 : ROWS + 2, :]
                        )
                        nc.sync.dma_start(
                            xg[:, ROWS + 2 :, :], xp[0, :, ROWS + 2 : GR + 2, :]
                        )
                    else:
                        nc.sync.dma_start(xg[:], xp[b, :, r0 : r0 + GR + 2, :])
                    # S correction for this group, host-replicated across
                    # partitions in DRAM: straight [128, GR*N] DMA
                    srep = srpool.tile([C, GR * N], F16, tag="srep")
                    nc.gpsimd.dma_start(srep[:], s9[b, g])

                    pss = [
                        pspool.tile([C, ROWS, N], F32, tag="ps", name=f"ps{j}")
                        for j in range(G)
                    ]

                    last = b == BPC - 1 and g == NBLK // G - 1
                    kh_s, kw_s = divmod(_SOLO, K)

                    def mm(j, p):
                        if p < len(_PAIRS):
                            t0 = _PAIRS[p][0]
                            bi = nc.tensor.matmul(
                                pss[j][:],
                                wt_t[:, t0 : t0 + 2, :],
                                _dr_rhs(xg, j, t0, deltas[p]),
                                start=(p == 0),
                                stop=False,
                                perf_mode=DR,
                            )
                        else:
                            bi = nc.tensor.matmul(
                                pss[j][:],
                                wt_t[:, _SOLO, :],
                                xg[
                                    :,
                                    ROWS * j + kh_s : ROWS * j + kh_s + ROWS,
                                    kw_s : kw_s + N,
                                ],
                                start=False,
                                stop=True,
                            )
                        _chain(bi)

                    def flush(j):
                        ob = opool.tile([C, ROWS, N], F32, tag="ob", name="ob")
                        nc.vector.scalar_tensor_tensor(
                            ob[:],
                            pss[j][:],
                            DESCALE,
                            srep[:, ROWS * N * j : ROWS * N * (j + 1)],
                            op0=mybir.AluOpType.mult,
                            op1=mybir.AluOpType.add,
                        )
                        eng = nc.scalar if (j % 2 and not last) else nc.sync
                        eng.dma_start(
                            out[b, :, r0 + ROWS * j : r0 + ROWS * (j + 1), :], ob[:]
                        )

                    if last:
                        # taps-inner: each block finishes (and flushes) early
                        # so only the final block's flush+DMA sits in the tail
                        for j in range(G):
                            for p in range(len(_PAIRS) + 1):
                                mm(j, p)
                            flush(j)
                    else:
                        for p in range(len(_PAIRS) + 1):
                            for j in range(G):
                                mm(j, p)
                        for j in range(G):
                            flush(j)


def _build_v9(nc, xp, wt, out, warmup=True, G=8):
    """fp8e4 DoubleRow conv, mean-correction moved to host post-processing.
    Per block: 4 DR pair-matmuls + 1 solo fp8 tap into one PSUM bank, then a
    single-input descale flush (out_f16 = psum * 2^-12) alternating between
    the Vector and Scalar engines so banks drain ~2x faster than the PE
    consumes them. Taps-outer order is forced so 8 consecutive matmuls share
    one DoubleRow weight load (stream rate ~222ns/mm)."""
    from concourse.tile_rust import add_dep_helper

    GR = G * ROWS
    F16 = mybir.dt.float16
    DR = mybir.MatmulPerfMode.DoubleRow
    deltas = [_pair_delta(t0, t1) for t0, t1 in _PAIRS]
    chain = [None]

    def _chain(bi):
        if chain[0] is not None:
            add_dep_helper(bi.ins, chain[0].ins, reason="taps-outer order")
        chain[0] = bi

    with tile.TileContext(nc) as tc:
        with (
            tc.tile_pool(name="xpool", bufs=3) as xpool,
            tc.tile_pool(name="wpool", bufs=1) as wpool,
            tc.tile_pool(name="opool", bufs=2 * G) as opool,
            tc.tile_pool(name="pspool", bufs=8, space="PSUM") as pspool,
        ):
            wt_t = wpool.tile([C, K * K, C], F8, tag="wt")
            nc.sync.dma_start(wt_t[:], wt[:])
            if warmup:
                scratch = wpool.tile([C, ROWS * N], F8, tag="scratch")
                nc.gpsimd.memset(scratch[:], 0.0)
                warm_ps = pspool.tile([C, ROWS, N], F32, tag="ps", name="warm_ps")
                for _ in range(8):
                    nc.tensor.matmul(
                        warm_ps[:], scratch[:, :C], scratch[:],
                        start=True, stop=True,
                    )

            for b in range(BPC):
                for g in range(NBLK // G):
                    r0 = GR * g
                    xg = xpool.tile([C, GR + 2, NP], F8, tag="xg")
                    if b == 0 and g == 0:
                        nc.sync.dma_start(
                            xg[:, 0 : ROWS + 2, :], xp[0, :, 0 : ROWS + 2, :]
                        )
                        nc.sync.dma_start(
                            xg[:, ROWS + 2 :, :], xp[0, :, ROWS + 2 : GR + 2, :]
                        )
                    else:
                        nc.sync.dma_start(xg[:], xp[b, :, r0 : r0 + GR + 2, :])

                    pss = [
                        pspool.tile([C, ROWS, N], F32, tag="ps", name=f"ps{j}")
                        for j in range(G)
                    ]
                    last = b == BPC - 1 and g == NBLK // G - 1
                    kh_s, kw_s = divmod(_SOLO, K)

                    def mm(j, p):
                        if p < len(_PAIRS):
                            t0 = _PAIRS[p][0]
                            bi = nc.tensor.matmul(
                                pss[j][:],
                                wt_t[:, t0 : t0 + 2, :],
                                _dr_rhs(xg, j, t0, deltas[p]),
                                start=(p == 0),
                                stop=False,
                                perf_mode=DR,
                            )
                        else:
                            bi = nc.tensor.matmul(
                                pss[j][:],
                                wt_t[:, _SOLO, :],
                                xg[
                                    :,
                                    ROWS * j + kh_s : ROWS * j + kh_s + ROWS,
                                    kw_s : kw_s + N,
                                ],
                                start=False,
                                stop=True,
                            )
                        _chain(bi)

                    def flush(j):
                        ob = opool.tile([C, ROWS, N], F16, tag="ob", name="ob")
                        if j % 2:
                            nc.scalar.activation(
                                ob[:],
                                pss[j][:],
                                mybir.ActivationFunctionType.Identity,
                                scale=DESCALE,
                            )
                        else:
                            nc.vector.tensor_scalar_mul(ob[:], pss[j][:], DESCALE)
                        eng = nc.scalar if (j % 2 and not last) else nc.sync
                        eng.dma_start(
                            out[b, :, r0 + ROWS * j : r0 + ROWS * (j + 1), :], ob[:]
                        )

                    if last:
                        for j in range(G):
                            for p in range(len(_PAIRS) + 1):
                                mm(j, p)
                            flush(j)
                    else:
                        for p in range(len(_PAIRS) + 1):
                            for j in range(G):
                                mm(j, p)
                        for j in range(G):
                            flush(j)


def _build_v1(nc, xp, wt, out, DT):
    """Whole-image input tiles; taps inner per block."""
    with tile.TileContext(nc) as tc:
        with (
            tc.tile_pool(name="xpool", bufs=2) as xpool,
            tc.tile_pool(name="wpool", bufs=1) as wpool,
            tc.tile_pool(name="opool", bufs=4) as opool,
            tc.tile_pool(name="pspool", bufs=8, space="PSUM") as pspool,
        ):
            wt_t = wpool.tile([C, K * K, C], DT, tag="wt")
            nc.sync.dma_start(wt_t[:], wt[:])

            for b in range(BPC):
                xp_t = xpool.tile([C, NP, NP], DT, tag="xp")
                nc.sync.dma_start(xp_t[:], xp[b])

                for r in range(NBLK):
                    ps = pspool.tile([C, ROWS, N], F32, tag="ps")
                    for t in range(K * K):
                        kh, kw = divmod(t, K)
                        rhs = xp_t[:, ROWS * r + kh : ROWS * r + kh + ROWS, kw : kw + N]
                        nc.tensor.matmul(
                            ps[:], wt_t[:, t, :], rhs,
                            start=(t == 0), stop=(t == K * K - 1),
                        )
                    ob = opool.tile([C, ROWS, N], F32, tag="ob")
                    nc.vector.tensor_copy(ob[:], ps[:])
                    nc.sync.dma_start(out[b, :, ROWS * r : ROWS * (r + 1), :], ob[:])


def _build_v2(nc, xp, wt, out, DT):
    """Per-block input tiles (ROWS+2 padded rows incl. halo) so compute
    starts after one small DMA; taps outer over groups of 8 blocks so 8
    consecutive matmuls share one weight load across 8 PSUM banks."""
    G = 8  # blocks per group = PSUM banks
    with tile.TileContext(nc) as tc:
        with (
            tc.tile_pool(name="xpool", bufs=2 * G) as xpool,
            tc.tile_pool(name="wpool", bufs=1) as wpool,
            tc.tile_pool(name="opool", bufs=G) as opool,
            tc.tile_pool(name="pspool", bufs=G, space="PSUM") as pspool,
        ):
            wt_t = wpool.tile([C, K * K, C], DT, tag="wt")
            nc.sync.dma_start(wt_t[:], wt[:])

            for b in range(BPC):
                for g in range(NBLK // G):
                    xb = []
                    for j in range(G):
                        r = g * G + j
                        xt = xpool.tile([C, ROWS + 2, NP], DT, tag="xb")
                        nc.sync.dma_start(
                            xt[:], xp[b, :, ROWS * r : ROWS * r + ROWS + 2, :]
                        )
                        xb.append(xt)
                    pss = [
                        pspool.tile([C, ROWS, N], F32, tag="ps", name=f"ps{j}")
                        for j in range(G)
                    ]
                    for t in range(K * K):
                        kh, kw = divmod(t, K)
                        for j in range(G):
                            nc.tensor.matmul(
                                pss[j][:],
                                wt_t[:, t, :],
                                xb[j][:, kh : kh + ROWS, kw : kw + N],
                                start=(t == 0),
                                stop=(t == K * K - 1),
                            )
                    for j in range(G):
                        r = g * G + j
                        ob = opool.tile([C, ROWS, N], F32, tag="ob")
                        nc.vector.tensor_copy(ob[:], pss[j][:])
                        nc.sync.dma_start(
                            out[b, :, ROWS * r : ROWS * (r + 1), :], ob[:]
                        )


def prep_inputs_v9(x: np.ndarray, kernel: np.ndarray):
    """fp8 prep without device-side S: returns (in_maps, s9) where s9 is the
    host-side mean-correction map 0.5*box9(channel-sum of x), [B, N, N] f32."""
    import ml_dtypes

    E4 = ml_dtypes.float8_e4m3
    x = np.asarray(x)
    kernel = np.asarray(kernel)
    xpad = np.zeros((B, C, NP, NP), E4)
    xpad[:, :, 1 : N + 1, 1 : N + 1] = np.clip(x * XS, -240, 240).astype(E4)
    wq = np.ascontiguousarray(
        ((kernel - 0.5) * WS).transpose(1, 2, 3, 0).reshape(C, K * K, C).astype(E4)
    )
    T = x.sum(axis=1, dtype=np.float32)  # [B, N, N]
    Tp = np.zeros((B, NP, NP), np.float32)
    Tp[:, 1 : N + 1, 1 : N + 1] = T
    s9 = np.zeros((B, N, N), np.float32)
    for kh in range(K):
        for kw in range(K):
            s9 += Tp[:, kh : kh + N, kw : kw + N]
    s9 *= 0.5
    in_maps = [
        {"xp": np.ascontiguousarray(xpad[i * BPC : (i + 1) * BPC]), "wt": wq}
        for i in range(NCORES)
    ]
    return in_maps, s9


def prep_inputs_v8(x: np.ndarray, kernel: np.ndarray):
    """fp8 prep: quantize x (scale 16) and mean-subtracted kernel (scale 256)
    to e4m3; precompute S = 0.5*box9(channel-sum of x) as f16."""
    import ml_dtypes

    E4 = ml_dtypes.float8_e4m3
    x = np.asarray(x)
    kernel = np.asarray(kernel)
    xpad = np.zeros((B, C, NP, NP), E4)
    xpad[:, :, 1 : N + 1, 1 : N + 1] = np.clip(x * XS, -240, 240).astype(E4)
    wq = np.ascontiguousarray(
        ((kernel - 0.5) * WS).transpose(1, 2, 3, 0).reshape(C, K * K, C).astype(E4)
    )
    T = x.sum(axis=1, dtype=np.float32)  # [B, N, N]
    Tp = np.zeros((B, NP, NP), np.float32)
    Tp[:, 1 : N + 1, 1 : N + 1] = T
    s9 = np.zeros((B, N, N), np.float32)
    for kh in range(K):
        for kw in range(K):
            s9 += Tp[:, kh : kh + N, kw : kw + N]
    GRP = N // 16  # groups of 16 rows per image
    s9 = (0.5 * s9).astype(np.float16).reshape(B, GRP, 1, 16 * N)
    s9r = np.ascontiguousarray(np.broadcast_to(s9, (B, GRP, C, 16 * N)))
    return [
        {
            "xp": np.ascontiguousarray(xpad[i * BPC : (i + 1) * BPC]),
            "wt": wq,
            "s9": s9r[i * BPC : (i + 1) * BPC],
        }
        for i in range(NCORES)
    ]


def prep_inputs(x: np.ndarray, kernel: np.ndarray, dtype: str = "f32r"):
    """Host-side prep: zero-pad x spatially, transpose kernel to [C_in, tap, C_out]."""
    npdt = _NPDT[dtype]
    x = np.asarray(x)
    kernel = np.asarray(kernel)
    xpad = np.zeros((B, C, NP, NP), dtype=npdt)
    xpad[:, :, 1 : N + 1, 1 : N + 1] = x
    # wt[c, kh*K+kw, o] = kernel[o, c, kh, kw]
    wt = np.ascontiguousarray(
        kernel.transpose(1, 2, 3, 0).reshape(C, K * K, C).astype(npdt)
    )
    in_maps = []
    for i in range(NCORES):
        in_maps.append(
            {
                "xp": np.ascontiguousarray(xpad[i * BPC : (i + 1) * BPC]),
                "wt": wt,
            }
        )
    return in_maps


def run(
    x: np.ndarray,
    kernel: np.ndarray,
    trace: bool = False,
    dtype: str = "f16",
    tmpdir: str | None = None,
    variant: str = "v4",
):
    """Build, compile, run on 8 cores; returns (out, BassKernelResults)."""
    from concourse.bass_utils import run_bass_kernel_spmd

    nc = build_nc(dtype=dtype, variant=variant)
    s9 = None
    if variant == "v9":
        in_maps, s9 = prep_inputs_v9(x, kernel)
    elif variant == "v8":
        in_maps = prep_inputs_v8(x, kernel)
    else:
        in_maps = prep_inputs(x, kernel, dtype=dtype)
    res = run_bass_kernel_spmd(
        nc, in_maps, core_ids=list(range(NCORES)), trace=trace, tmpdir=tmpdir
    )
    out = np.concatenate([res.results[i]["out"] for i in range(NCORES)], axis=0)
    if s9 is not None:
        # add back the host-side mean-correction (w = (w-0.5) + 0.5 split)
        out = out.astype(np.float32) + s9[:, None, :, :]
    return out, res


def kernel(x: np.ndarray, kernel: np.ndarray) -> np.ndarray:
    out, _ = run(x, kernel, trace=False, dtype="f16", variant="v4")
    return out



# revision 32
# speedup vs baseline: 1.1788x; 1.0120x over previous
"""Trainium2 Bass kernel for nn_BlockConv_10514079941182.

3x3 SAME conv: x[32,128,128,128] (NCHW) * kernel[128,128,3,3] (OIHW)
-> out[32,128,128,128], fp32.

Strategy: data-parallel over batch across 8 NeuronCores (4 images/core),
no collectives. Per image, x is host-padded to [C=128, 130, 130] and
held in SBUF with C_in as the partition dim. The conv is 9 accumulating
PE matmuls per 4-row output block: contraction over C_in (partition
dim), weights [C_in, C_out] stationary, shifted windows of the padded
image as the moving operand (free size 4*128=512 = one full PSUM bank).

dtype options (matmul moving/stationary; PSUM accumulates fp32 always):
- f16 (default): full PE rate AND the 2-byte FWL-eligible weight load
  hides under each matmul -> ~218 ns/matmul cadence, ~2.8e-4 rel err.
- f32r: reduced-precision fp32 (TF32-like), full PE rate at free>=256,
  ~1.4e-4 rel err, but the 4-byte per-matmul weight reload is partially
  exposed -> ~237 ns/matmul (~8% slower overall).
- f32: true fp32, 4 cycles/row (~3.6x slower). Unused.

Measured (NTFF profile, core 0): ~272-273 us HW exec for the full
per-core workload (1152 matmuls of [128x128]@[128x512]), ~91% of the
PE streaming roofline incl. fixed ~7.5us preamble + ~10.5us drain tail.
"""

import sys

for _p in ("/opt/trn_rl_repo", "/root/.axon_site/_ro/trn_rl_repo"):
    if _p not in sys.path:
        sys.path.append(_p)

import numpy as np

import concourse.bacc as bacc
import concourse.bass as bass
import concourse.mybir as mybir
import concourse.tile as tile
import concourse.bass_utils as _bu

def _dedupe_ldweights(nc):
    """With taps-outer loop order, 8 consecutive matmuls share one stationary
    tensor; delete the repeated standalone InstLdweights that tile
    legalization emits per matmul (weights persist in the PE array)."""
    n = 0
    for blk in nc.main_func.blocks:
        prev_key = None
        drop = []
        for inst in blk.instructions:
            if isinstance(inst, mybir.InstLdweights):
                si = inst.sync_info
                has_sem = si is not None and (len(si.on_wait) or len(si.on_update))
                key = (str(inst.ins[0]), str(inst.perf_mode), str(inst.tile_position))
                if key == prev_key and not has_sem:
                    drop.append(inst)
                    n += 1
                else:
                    prev_key = key
            elif isinstance(inst, mybir.InstMatmult):
                pass  # matmuls don't invalidate the loaded weights
        for inst in drop:
            blk.instructions.remove(inst)
    return n

B, C, N, K = 32, 128, 128, 3
NCORES = 8
BPC = B // NCORES  # images per core
NP = N + 2  # padded spatial size
ROWS = 4  # output rows per matmul block (4*128 = 512 free = 1 PSUM bank)
NBLK = N // ROWS

F32 = mybir.dt.float32
F32R = mybir.dt.float32r
F16 = mybir.dt.float16

_DT = {"f32r": F32R, "f32": F32, "f16": F16}
_NPDT = {"f32r": np.float32, "f32": np.float32, "f16": np.float16}


def build_nc(dtype: str = "f32r", variant: str = "v2") -> bass.Bass:
    """Build the SPMD per-core program (same on all 8 cores)."""
    nc = bacc.Bacc("TRN2", target_bir_lowering=False, debug=False)

    # float32r = reduced-precision fp32 matmul dtype: full PE rate at
    # free-dim >= 256 (vs 4x slower for true fp32), ~1.5e-4 rel err.
    # The BIR verifier requires the whole producer chain to be f32r.
    # float16: same PE rate, ~2.8e-4 rel err, but the 2-byte weight load
    # (FWL-eligible) hides under the matmul, unlike the fp32 one.
    if variant == "v9":
        xp = nc.dram_tensor("xp", [BPC, C, NP, NP], F8, kind="ExternalInput")
        wt = nc.dram_tensor("wt", [C, K * K, C], F8, kind="ExternalInput")
        out = nc.dram_tensor("out", [BPC, C, N, N], F16, kind="ExternalOutput")
        _build_v9(nc, xp, wt, out)
        nc.compile()
        return nc

    if variant == "v8":
        GRP = NBLK // 4  # groups per image (G=4 blocks each)
        xp = nc.dram_tensor("xp", [BPC, C, NP, NP], F8, kind="ExternalInput")
        wt = nc.dram_tensor("wt", [C, K * K, C], F8, kind="ExternalInput")
        s9 = nc.dram_tensor("s9", [BPC, GRP, C, 4 * ROWS * N], F16, kind="ExternalInput")
        out = nc.dram_tensor("out", [BPC, C, N, N], F32, kind="ExternalOutput")
        _build_v8(nc, xp, wt, out, s9)
        if _DEDUPE_LDW:
            _dedupe_ldweights(nc)
        nc.compile()
        return nc

    DT = _DT[dtype]
    xp = nc.dram_tensor("xp", [BPC, C, NP, NP], DT, kind="ExternalInput")
    wt = nc.dram_tensor("wt", [C, K * K, C], DT, kind="ExternalInput")
    out = nc.dram_tensor("out", [BPC, C, N, N], F32, kind="ExternalOutput")

    nc._taps_inner = variant == "v5"
    if variant == "v1":
        _build_v1(nc, xp, wt, out, DT)
    elif variant == "v2":
        _build_v2(nc, xp, wt, out, DT)
    elif variant == "v3":
        _build_v3(nc, xp, wt, out, DT)
    else:
        _build_v3(nc, xp, wt, out, DT, warmup=True, psum_tail_dma=True)
    nc.compile()
    return nc


def _build_v3(nc, xp, wt, out, DT, warmup=False, psum_tail_dma=False):
    """v2 + group-level input DMAs (4/image instead of 32) to cut the
    serial DMA-enqueue chain, a split first DMA so matmuls start after
    ~6 rows, and output DMAs alternating sync/scalar queues.
    warmup: dummy matmuls on a zeroed scratch tile during the DMA lead-in
    so the HAM clock-gate is already at 8/8 when real matmuls start.
    psum_tail_dma: DMA the final group's PSUM banks straight to DRAM,
    skipping the serial DVE copy chain in the kernel tail."""
    G = 8
    GR = G * ROWS  # 32 rows per group
    with tile.TileContext(nc) as tc:
        with (
            tc.tile_pool(name="xpool", bufs=3) as xpool,
            tc.tile_pool(name="wpool", bufs=1) as wpool,
            tc.tile_pool(name="opool", bufs=G) as opool,
            tc.tile_pool(name="pspool", bufs=G, space="PSUM") as pspool,
        ):
            wt_t = wpool.tile([C, K * K, C], DT, tag="wt")
            # weights go on sync's HWDGE queue FIRST: the gpsimd (SWDGE)
            # path measured ~4us slower start+transfer and gated the first
            # real matmul at 15us instead of ~9us
            nc.sync.dma_start(wt_t[:], wt[:])
            if warmup:
                # full-width (N=512) dummy matmuls covering the ~2us input-DMA
                # wait: they start the HAM clock-gate warm-up early without
                # delaying the first real matmul (PE runs them first in order)
                scratch = wpool.tile([C, ROWS * N], DT, tag="scratch")
                nc.gpsimd.memset(scratch[:], 0.0)
                warm_ps = pspool.tile([C, ROWS, N], F32, tag="ps", name="warm_ps")
                for _ in range(8):
                    nc.tensor.matmul(
                        warm_ps[:], scratch[:, :C], scratch[:],
                        start=True, stop=True,
                    )

            for b in range(BPC):
                for g in range(NBLK // G):
                    r0 = GR * g
                    last = b == BPC - 1 and g == NBLK // G - 1
                    xg = xpool.tile([C, GR + 2, NP], DT, tag="xg")
                    if b == 0 and g == 0:
                        # split: block 0's rows land first so the PE can start
                        nc.sync.dma_start(
                            xg[:, 0 : ROWS + 2, :], xp[0, :, 0 : ROWS + 2, :]
                        )
                        nc.sync.dma_start(
                            xg[:, ROWS + 2 :, :], xp[0, :, ROWS + 2 : GR + 2, :]
                        )
                    else:
                        nc.sync.dma_start(xg[:], xp[b, :, r0 : r0 + GR + 2, :])
                    pss = [
                        pspool.tile([C, ROWS, N], F32, tag="ps", name=f"ps{j}")
                        for j in range(G)
                    ]

                    def mm(j, t):
                        kh, kw = divmod(t, K)
                        nc.tensor.matmul(
                            pss[j][:],
                            wt_t[:, t, :],
                            xg[:, ROWS * j + kh : ROWS * j + kh + ROWS, kw : kw + N],
                            start=(t == 0),
                            stop=(t == K * K - 1),
                        )

                    def flush(j):
                        ob = opool.tile([C, ROWS, N], F32, tag="ob", name="ob")
                        nc.vector.tensor_copy(ob[:], pss[j][:])
                        # scalar dma_start = slow SWDGE (~76 GB/s): fine
                        # mid-stream where transfers hide under compute, but
                        # the final group must drain fast on sync's HWDGE or
                        # its last transfer (~3.4us) sits in the kernel tail
                        eng = nc.scalar if (j % 2 and not last) else nc.sync
                        eng.dma_start(
                            out[b, :, r0 + ROWS * j : r0 + ROWS * (j + 1), :], ob[:]
                        )

                    if (last and psum_tail_dma) or nc._taps_inner:
                        # taps-inner: each bank finishes (and flushes) early;
                        # only the final block's copy+DMA lands in the tail
                        for j in range(G):
                            for t in range(K * K):
                                mm(j, t)
                            flush(j)
                    else:
                        for t in range(K * K):
                            for j in range(G):
                                mm(j, t)
                        for j in range(G):
                            flush(j)


F8 = mybir.dt.float8e4
_DEDUPE_LDW = False
XS = 16.0  # x fp8 scale
WS = 256.0  # w' fp8 scale
DESCALE = 1.0 / (XS * WS)

# tap pairing for DoubleRow: taps row-major (kh*3+kw); pairs (0,1),(2,3),
# (4,5),(6,7), solo tap 8. delta = SBUF element offset between the pair's
# shifted windows of the padded image tile.
_PAIRS = [(0, 1), (2, 3), (4, 5), (6, 7)]
_SOLO = 8


def _pair_delta(t0, t1):
    kh0, kw0 = divmod(t0, K)
    kh1, kw1 = divmod(t1, K)
    return (kh1 - kh0) * NP + (kw1 - kw0)


def _dr_rhs(xg, j, t0, delta):
    """Moving AP [C, 2, ROWS, N]: pair axis = two shifted windows (stride
    delta) of the padded group tile xg[C, GR+2, NP]."""
    kh0, kw0 = divmod(t0, K)
    ap = xg[:, ROWS * j + kh0 : ROWS * j + kh0 + ROWS, kw0 : kw0 + N].unsqueeze(1)
    v = ap.ap
    V = type(v)
    dims = [list(d) for d in v]
    dims[1] = [delta, 2]
    ap.ap = V(dims)
    return ap


def _build_v8(nc, xp, wt, out, s9, warmup=True, G=4):
    """fp8e4 DoubleRow conv: per block 4 DR pair-matmuls + 1 solo fp8 tap,
    weights mean-subtracted on host; the 0.5*box9(channel-sum) correction
    S is host-replicated and added in the DVE flush (scalar_tensor_tensor:
    out = psum*DESCALE + S). Groups of G=4 blocks alternate between the two
    halves of the 8 PSUM banks so a group's flushes overlap the next
    group's matmuls instead of stalling its start=True taps."""
    from concourse.tile_rust import add_dep_helper

    GR = G * ROWS  # rows per group
    F16 = mybir.dt.float16
    DR = mybir.MatmulPerfMode.DoubleRow
    deltas = [_pair_delta(t0, t1) for t0, t1 in _PAIRS]
    chain = [None]

    def _chain(bi):
        # total order over all matmuls: keeps the scheduler in taps-outer
        # order so consecutive matmuls share one weight load
        if chain[0] is not None:
            add_dep_helper(bi.ins, chain[0].ins, reason="taps-outer order")
        chain[0] = bi
    with tile.TileContext(nc) as tc:
        with (
            tc.tile_pool(name="xpool", bufs=3) as xpool,
            tc.tile_pool(name="wpool", bufs=1) as wpool,
            tc.tile_pool(name="srpool", bufs=3) as srpool,
            tc.tile_pool(name="opool", bufs=8) as opool,
            tc.tile_pool(name="pspool", bufs=8, space="PSUM") as pspool,
        ):
            wt_t = wpool.tile([C, K * K, C], F8, tag="wt")
            nc.sync.dma_start(wt_t[:], wt[:])
            if warmup:
                scratch = wpool.tile([C, ROWS * N], F8, tag="scratch")
                nc.gpsimd.memset(scratch[:], 0.0)
                warm_ps = pspool.tile([C, ROWS, N], F32, tag="ps", name="warm_ps")
                for _ in range(8):
                    nc.tensor.matmul(
                        warm_ps[:], scratch[:, :C], scratch[:],
                        start=True, stop=True,
                    )

            for b in range(BPC):
                for g in range(NBLK // G):
                    r0 = GR * g
                    xg = xpool.tile([C, GR + 2, NP], F8, tag="xg")
                    if b == 0 and g == 0:
                        nc.sync.dma_start(
                            xg[:, 0 : ROWS + 2, :], xp[0, :, 0 : ROWS + 2, :]
                        )
                        nc.sync.dma_start(
                            xg[:, ROWS + 2 :, :], xp[0, :, ROWS + 2 : GR + 2, :]
                        )
                    else:
                        nc.sync.dma_start(xg[:], xp[b, :, r0 : r0 + GR + 2, :])
                    # S correction for this group, host-replicated across
                    # partitions in DRAM: straight [128, GR*N] DMA
                    srep = srpool.tile([C, GR * N], F16, tag="srep")
                    nc.gpsimd.dma_start(srep[:], s9[b, g])

                    pss = [
                        pspool.tile([C, ROWS, N], F32, tag="ps", name=f"ps{j}")
                        for j in range(G)
                    ]

                    last = b == BPC - 1 and g == NBLK // G - 1
                    kh_s, kw_s = divmod(_SOLO, K)

                    def mm(j, p):
                        if p < len(_PAIRS):
                            t0 = _PAIRS[p][0]
                            bi = nc.tensor.matmul(
                                pss[j][:],
                                wt_t[:, t0 : t0 + 2, :],
                                _dr_rhs(xg, j, t0, deltas[p]),
                                start=(p == 0),
                                stop=False,
                                perf_mode=DR,
                            )
                        else:
                            bi = nc.tensor.matmul(
                                pss[j][:],
                                wt_t[:, _SOLO, :],
                                xg[
                                    :,
                                    ROWS * j + kh_s : ROWS * j + kh_s + ROWS,
                                    kw_s : kw_s + N,
                                ],
                                start=False,
                                stop=True,
                            )
                        _chain(bi)

                    def flush(j):
                        ob = opool.tile([C, ROWS, N], F32, tag="ob", name="ob")
                        nc.vector.scalar_tensor_tensor(
                            ob[:],
                            pss[j][:],
                            DESCALE,
                            srep[:, ROWS * N * j : ROWS * N * (j + 1)],
                            op0=mybir.AluOpType.mult,
                            op1=mybir.AluOpType.add,
                        )
                        eng = nc.scalar if (j % 2 and not last) else nc.sync
                        eng.dma_start(
                            out[b, :, r0 + ROWS * j : r0 + ROWS * (j + 1), :], ob[:]
                        )

                    if last:
                        # taps-inner: each block finishes (and flushes) early
                        # so only the final block's flush+DMA sits in the tail
                        for j in range(G):
                            for p in range(len(_PAIRS) + 1):
                                mm(j, p)
                            flush(j)
                    else:
                        for p in range(len(_PAIRS) + 1):
                            for j in range(G):
                                mm(j, p)
                        for j in range(G):
                            flush(j)


def _build_v9(nc, xp, wt, out, warmup=True, G=8):
    """fp8e4 DoubleRow conv, mean-correction moved to host post-processing.
    Per block: 4 DR pair-matmuls + 1 solo fp8 tap into one PSUM bank, then a
    single-input descale flush (out_f16 = psum * 2^-12) alternating between
    the Vector and Scalar engines so banks drain ~2x faster than the PE
    consumes them. Taps-outer order is forced so 8 consecutive matmuls share
    one DoubleRow weight load (stream rate ~222ns/mm)."""
    from concourse.tile_rust import add_dep_helper

    GR = G * ROWS
    F16 = mybir.dt.float16
    DR = mybir.MatmulPerfMode.DoubleRow
    deltas = [_pair_delta(t0, t1) for t0, t1 in _PAIRS]
    chain = [None]

    def _chain(bi):
        if chain[0] is not None:
            add_dep_helper(bi.ins, chain[0].ins, reason="taps-outer order")
        chain[0] = bi

    with tile.TileContext(nc) as tc:
        with (
            tc.tile_pool(name="xpool", bufs=4) as xpool,
            tc.tile_pool(name="wpool", bufs=1) as wpool,
            tc.tile_pool(name="opool", bufs=4 * G) as opool,
            tc.tile_pool(name="pspool", bufs=8, space="PSUM") as pspool,
        ):
            wt_t = wpool.tile([C, K * K, C], F8, tag="wt")
            nc.sync.dma_start(wt_t[:], wt[:])
            if warmup:
                scratch = wpool.tile([C, ROWS * N], F8, tag="scratch")
                nc.gpsimd.memset(scratch[:], 0.0)
                warm_ps = pspool.tile([C, ROWS, N], F32, tag="ps", name="warm_ps")
                for _ in range(8):
                    nc.tensor.matmul(
                        warm_ps[:], scratch[:, :C], scratch[:],
                        start=True, stop=True,
                    )

            for b in range(BPC):
                for g in range(NBLK // G):
                    r0 = GR * g
                    xg = xpool.tile([C, GR + 2, NP], F8, tag="xg")
                    if b == 0 and g == 0:
                        nc.sync.dma_start(
                            xg[:, 0 : ROWS + 2, :], xp[0, :, 0 : ROWS + 2, :]
                        )
                        nc.sync.dma_start(
                            xg[:, ROWS + 2 :, :], xp[0, :, ROWS + 2 : GR + 2, :]
                        )
                    else:
                        nc.sync.dma_start(xg[:], xp[b, :, r0 : r0 + GR + 2, :])

                    pss = [
                        pspool.tile([C, ROWS, N], F32, tag="ps", name=f"ps{j}")
                        for j in range(G)
                    ]
                    last = b == BPC - 1 and g == NBLK // G - 1
                    kh_s, kw_s = divmod(_SOLO, K)

                    def mm(j, p):
                        if p < len(_PAIRS):
                            t0 = _PAIRS[p][0]
                            bi = nc.tensor.matmul(
                                pss[j][:],
                                wt_t[:, t0 : t0 + 2, :],
                                _dr_rhs(xg, j, t0, deltas[p]),
                                start=(p == 0),
                                stop=False,
                                perf_mode=DR,
                            )
                        else:
                            bi = nc.tensor.matmul(
                                pss[j][:],
                                wt_t[:, _SOLO, :],
                                xg[
                                    :,
                                    ROWS * j + kh_s : ROWS * j + kh_s + ROWS,
                                    kw_s : kw_s + N,
                                ],
                                start=False,
                                stop=True,
                            )
                        _chain(bi)

                    def flush(j):
                        ob = opool.tile([C, ROWS, N], F16, tag="ob", name="ob")
                        if j % 2:
                            nc.scalar.activation(
                                ob[:],
                                pss[j][:],
                                mybir.ActivationFunctionType.Identity,
                                scale=DESCALE,
                            )
                        else:
                            nc.vector.tensor_scalar_mul(ob[:], pss[j][:], DESCALE)
                        eng = nc.scalar if (j % 2 and not last) else nc.sync
                        eng.dma_start(
                            out[b, :, r0 + ROWS * j : r0 + ROWS * (j + 1), :], ob[:]
                        )

                    if last:
                        for j in range(G):
                            for p in range(len(_PAIRS) + 1):
                                mm(j, p)
                            flush(j)
                    else:
                        for p in range(len(_PAIRS) + 1):
                            for j in range(G):
                                mm(j, p)
                        for j in range(G):
                            flush(j)


def _build_v1(nc, xp, wt, out, DT):
    """Whole-image input tiles; taps inner per block."""
    with tile.TileContext(nc) as tc:
        with (
            tc.tile_pool(name="xpool", bufs=2) as xpool,
            tc.tile_pool(name="wpool", bufs=1) as wpool,
            tc.tile_pool(name="opool", bufs=4) as opool,
            tc.tile_pool(name="pspool", bufs=8, space="PSUM") as pspool,
        ):
            wt_t = wpool.tile([C, K * K, C], DT, tag="wt")
            nc.sync.dma_start(wt_t[:], wt[:])

            for b in range(BPC):
                xp_t = xpool.tile([C, NP, NP], DT, tag="xp")
                nc.sync.dma_start(xp_t[:], xp[b])

                for r in range(NBLK):
                    ps = pspool.tile([C, ROWS, N], F32, tag="ps")
                    for t in range(K * K):
                        kh, kw = divmod(t, K)
                        rhs = xp_t[:, ROWS * r + kh : ROWS * r + kh + ROWS, kw : kw + N]
                        nc.tensor.matmul(
                            ps[:], wt_t[:, t, :], rhs,
                            start=(t == 0), stop=(t == K * K - 1),
                        )
                    ob = opool.tile([C, ROWS, N], F32, tag="ob")
                    nc.vector.tensor_copy(ob[:], ps[:])
                    nc.sync.dma_start(out[b, :, ROWS * r : ROWS * (r + 1), :], ob[:])


def _build_v2(nc, xp, wt, out, DT):
    """Per-block input tiles (ROWS+2 padded rows incl. halo) so compute
    starts after one small DMA; taps outer over groups of 8 blocks so 8
    consecutive matmuls share one weight load across 8 PSUM banks."""
    G = 8  # blocks per group = PSUM banks
    with tile.TileContext(nc) as tc:
        with (
            tc.tile_pool(name="xpool", bufs=2 * G) as xpool,
            tc.tile_pool(name="wpool", bufs=1) as wpool,
            tc.tile_pool(name="opool", bufs=G) as opool,
            tc.tile_pool(name="pspool", bufs=G, space="PSUM") as pspool,
        ):
            wt_t = wpool.tile([C, K * K, C], DT, tag="wt")
            nc.sync.dma_start(wt_t[:], wt[:])

            for b in range(BPC):
                for g in range(NBLK // G):
                    xb = []
                    for j in range(G):
                        r = g * G + j
                        xt = xpool.tile([C, ROWS + 2, NP], DT, tag="xb")
                        nc.sync.dma_start(
                            xt[:], xp[b, :, ROWS * r : ROWS * r + ROWS + 2, :]
                        )
                        xb.append(xt)
                    pss = [
                        pspool.tile([C, ROWS, N], F32, tag="ps", name=f"ps{j}")
                        for j in range(G)
                    ]
                    for t in range(K * K):
                        kh, kw = divmod(t, K)
                        for j in range(G):
                            nc.tensor.matmul(
                                pss[j][:],
                                wt_t[:, t, :],
                                xb[j][:, kh : kh + ROWS, kw : kw + N],
                                start=(t == 0),
                                stop=(t == K * K - 1),
                            )
                    for j in range(G):
                        r = g * G + j
                        ob = opool.tile([C, ROWS, N], F32, tag="ob")
                        nc.vector.tensor_copy(ob[:], pss[j][:])
                        nc.sync.dma_start(
                            out[b, :, ROWS * r : ROWS * (r + 1), :], ob[:]
                        )


def prep_inputs_v9(x: np.ndarray, kernel: np.ndarray):
    """fp8 prep without device-side S: returns (in_maps, s9) where s9 is the
    host-side mean-correction map 0.5*box9(channel-sum of x), [B, N, N] f32."""
    import ml_dtypes

    E4 = ml_dtypes.float8_e4m3
    x = np.asarray(x)
    kernel = np.asarray(kernel)
    xpad = np.zeros((B, C, NP, NP), E4)
    xpad[:, :, 1 : N + 1, 1 : N + 1] = np.clip(x * XS, -240, 240).astype(E4)
    wq = np.ascontiguousarray(
        ((kernel - 0.5) * WS).transpose(1, 2, 3, 0).reshape(C, K * K, C).astype(E4)
    )
    T = x.sum(axis=1, dtype=np.float32)  # [B, N, N]
    Tp = np.zeros((B, NP, NP), np.float32)
    Tp[:, 1 : N + 1, 1 : N + 1] = T
    s9 = np.zeros((B, N, N), np.float32)
    for kh in range(K):
        for kw in range(K):
            s9 += Tp[:, kh : kh + N, kw : kw + N]
    s9 *= 0.5
    in_maps = [
        {"xp": np.ascontiguousarray(xpad[i * BPC : (i + 1) * BPC]), "wt": wq}
        for i in range(NCORES)
    ]
    return in_maps, s9


def prep_inputs_v8(x: np.ndarray, kernel: np.ndarray):
    """fp8 prep: quantize x (scale 16) and mean-subtracted kernel (scale 256)
    to e4m3; precompute S = 0.5*box9(channel-sum of x) as f16."""
    import ml_dtypes

    E4 = ml_dtypes.float8_e4m3
    x = np.asarray(x)
    kernel = np.asarray(kernel)
    xpad = np.zeros((B, C, NP, NP), E4)
    xpad[:, :, 1 : N + 1, 1 : N + 1] = np.clip(x * XS, -240, 240).astype(E4)
    wq = np.ascontiguousarray(
        ((kernel - 0.5) * WS).transpose(1, 2, 3, 0).reshape(C, K * K, C).astype(E4)
    )
    T = x.sum(axis=1, dtype=np.float32)  # [B, N, N]
    Tp = np.zeros((B, NP, NP), np.float32)
    Tp[:, 1 : N + 1, 1 : N + 1] = T
    s9 = np.zeros((B, N, N), np.float32)
    for kh in range(K):
        for kw in range(K):
            s9 += Tp[:, kh : kh + N, kw : kw + N]
    GRP = N // 16  # groups of 16 rows per image
    s9 = (0.5 * s9).astype(np.float16).reshape(B, GRP, 1, 16 * N)
    s9r = np.ascontiguousarray(np.broadcast_to(s9, (B, GRP, C, 16 * N)))
    return [
        {
            "xp": np.ascontiguousarray(xpad[i * BPC : (i + 1) * BPC]),
            "wt": wq,
            "s9": s9r[i * BPC : (i + 1) * BPC],
        }
        for i in range(NCORES)
    ]


def prep_inputs(x: np.ndarray, kernel: np.ndarray, dtype: str = "f32r"):
    """Host-side prep: zero-pad x spatially, transpose kernel to [C_in, tap, C_out]."""
    npdt = _NPDT[dtype]
    x = np.asarray(x)
    kernel = np.asarray(kernel)
    xpad = np.zeros((B, C, NP, NP), dtype=npdt)
    xpad[:, :, 1 : N + 1, 1 : N + 1] = x
    # wt[c, kh*K+kw, o] = kernel[o, c, kh, kw]
    wt = np.ascontiguousarray(
        kernel.transpose(1, 2, 3, 0).reshape(C, K * K, C).astype(npdt)
    )
    in_maps = []
    for i in range(NCORES):
        in_maps.append(
            {
                "xp": np.ascontiguousarray(xpad[i * BPC : (i + 1) * BPC]),
                "wt": wt,
            }
        )
    return in_maps


def run(
    x: np.ndarray,
    kernel: np.ndarray,
    trace: bool = False,
    dtype: str = "f16",
    tmpdir: str | None = None,
    variant: str = "v4",
):
    """Build, compile, run on 8 cores; returns (out, BassKernelResults)."""
    from concourse.bass_utils import run_bass_kernel_spmd

    nc = build_nc(dtype=dtype, variant=variant)
    s9 = None
    if variant == "v9":
        in_maps, s9 = prep_inputs_v9(x, kernel)
    elif variant == "v8":
        in_maps = prep_inputs_v8(x, kernel)
    else:
        in_maps = prep_inputs(x, kernel, dtype=dtype)
    res = run_bass_kernel_spmd(
        nc, in_maps, core_ids=list(range(NCORES)), trace=trace, tmpdir=tmpdir
    )
    out = np.concatenate([res.results[i]["out"] for i in range(NCORES)], axis=0)
    if s9 is not None:
        # add back the host-side mean-correction (w = (w-0.5) + 0.5 split)
        out = out.astype(np.float32) + s9[:, None, :, :]
    return out, res


def kernel(x: np.ndarray, kernel: np.ndarray) -> np.ndarray:
    out, _ = run(x, kernel, trace=False, dtype="f16", variant="v4")
    return out

